# revision 5
# baseline (speedup 1.0000x reference)
"""MoE top-2 routing kernel for 8 Trainium2 NeuronCores.

Strategy (expert parallelism per the sharding hint):
  Launch A (data-parallel gate): each core computes softmax gate + top-2
    combine weights for its 1024-token slice, on device (fp32 matmul for
    exact top-k selection).
  Host: builds per-expert token index lists from the combine weights
    (routing bookkeeping only - all math stays on device).
  Launch B (expert-parallel): core i owns expert i. Gathers its tokens'
    rows of x by index (indirect DMA), transposes on the PE, runs the
    grouped GEMM against its resident expert weight in fp32r, seeds PSUM
    with the expert bias via a K=1 ones-matmul, scales rows by the gate
    probability, and writes the compact [C, 2048] result.
  Host: scatter-adds each expert's compact output into the full [B, 2048]
    output (each token appears in exactly two experts' lists).
"""

import numpy as np

import concourse.bass as bass
import concourse.mybir as mybir
from concourse.bass_utils import run_bass_kernel_spmd
from concourse.masks import make_identity
from concourse.tile import TileContext

B = 8192
D = 2048
O = 2048
E = 8
P = 128
C = 2304  # per-expert token capacity (18 chunks of 128; actual max load 2193)
BS = B // E  # tokens per core in the gate launch

f32 = mybir.dt.float32
f32r = mybir.dt.float32r
i32 = mybir.dt.int32


MAXW = 1  # this walrus build accepts one sync-wait command per instruction
_wsctr = [0]


def split_excess_waits(nc):
    """Post-pass: any instruction carrying more than MAXW sem-waits gets the
    excess moved onto spliced same-engine NoOps just before it (same-engine
    ge-waits executed earlier are semantically identical)."""
    import bass_rust

    for f in nc.m.functions:
        for blk in f.blocks:
            out = []
            changed = False
            for inst in blk.instructions:
                si = inst.sync_info
                if si is not None and len(si.on_wait) > MAXW:
                    waits = list(si.on_wait)
                    excess, keep = waits[:-MAXW], waits[-MAXW:]
                    for i in range(0, len(excess), MAXW):
                        _wsctr[0] += 1
                        nop = bass_rust.InstNoOp(
                            name=f"WSPLIT-{_wsctr[0]}", ins=[], outs=[]
                        )
                        nop.engine = inst.engine
                        nop.sync_info = mybir.SyncInfo(
                            on_wait=excess[i : i + MAXW], on_update=[]
                        )
                        out.append(nop)
                    inst.sync_info = mybir.SyncInfo(
                        on_wait=keep, on_update=list(si.on_update)
                    )
                    changed = True
                out.append(inst)
            if changed:
                blk.instructions = out


def build_gate_kernel():
    """Per core: gate for its BS-token slice. In: xT slice [D, BS], W_g
    [D, E], b_g [E, 1]. Out: combine weights c [BS, E] (top-2 masked
    softmax probs, zeros elsewhere)."""
    nc = bass.Bass()
    xt = nc.dram_tensor("xt", [D, BS], f32, kind="ExternalInput")
    wg = nc.dram_tensor("wg", [D, E], f32, kind="ExternalInput")
    bg = nc.dram_tensor("bg", [E, 1], f32, kind="ExternalInput")
    cout = nc.dram_tensor("c", [BS, E], f32, kind="ExternalOutput")
    NB = BS // 512

    with TileContext(nc) as tc:
        with (
            tc.tile_pool(name="const", bufs=1) as cpool,
            tc.tile_pool(name="work", bufs=2) as wpool,
            tc.tile_pool(name="psum", bufs=2, space="PSUM") as ppool,
            tc.tile_pool(name="psumg", bufs=4, space="PSUM") as pgpool,
        ):
            ident = cpool.tile([P, P], f32)
            make_identity(nc, ident[:])
            wgs = cpool.tile([P, 16, E], f32)
            nc.sync.dma_start(
                out=wgs[:], in_=wg.rearrange("(kt p) e -> p kt e", p=P)
            )
            bgs = cpool.tile([E, 1], f32)
            nc.sync.dma_start(out=bgs[:], in_=bg[:, :])

            for bc in range(NB):
                xts = wpool.tile([P, 16, 512], f32, tag="xts")
                nc.sync.dma_start(
                    out=xts[:],
                    in_=xt[:, bc * 512 : (bc + 1) * 512].rearrange(
                        "(kt p) b -> p kt b", p=P
                    ),
                )
                gt_ps = ppool.tile([E, 512], f32, tag="gt_ps")
                for k in range(16):
                    nc.tensor.matmul(
                        gt_ps[:],
                        lhsT=wgs[:, k, :],
                        rhs=xts[:, k, :],
                        start=(k == 0),
                        stop=(k == 15),
                    )
                gt = wpool.tile([E, 512], f32, tag="gt")
                # copy out of PSUM and add the gate bias (per-partition here)
                nc.vector.tensor_scalar_add(gt[:], gt_ps[:], bgs[:, 0:1])

                for t in range(4):
                    g_ps = pgpool.tile([P, E], f32, tag="g_ps")
                    nc.tensor.transpose(
                        out=g_ps[:],
                        in_=gt[:, t * P : (t + 1) * P],
                        identity=ident[:E, :E],
                    )
                    g = wpool.tile([P, E], f32, tag="g")
                    nc.vector.tensor_copy(g[:], g_ps[:])
                    mx = wpool.tile([P, 8], f32, tag="mx")
                    nc.vector.max(out=mx[:], in_=g[:])
                    nc.vector.tensor_scalar(
                        g[:], g[:], mx[:, 0:1], None, op0=mybir.AluOpType.subtract
                    )
                    nc.scalar.activation(g[:], g[:], mybir.ActivationFunctionType.Exp)
                    s = wpool.tile([P, 1], f32, tag="s")
                    nc.vector.reduce_sum(out=s[:], in_=g[:], axis=mybir.AxisListType.X)
                    r = wpool.tile([P, 1], f32, tag="r")
                    nc.vector.reciprocal(r[:], s[:])
                    nc.vector.tensor_scalar_mul(g[:], g[:], r[:, 0:1])
                    # top-2 mask: prob >= (second largest prob)
                    mx2 = wpool.tile([P, 8], f32, tag="mx2")
                    nc.vector.max(out=mx2[:], in_=g[:])
                    msk = wpool.tile([P, E], f32, tag="msk")
                    nc.vector.tensor_scalar(
                        msk[:], g[:], mx2[:, 1:2], None, op0=mybir.AluOpType.is_ge
                    )
                    cc = wpool.tile([P, E], f32, tag="cc")
                    nc.vector.tensor_mul(cc[:], g[:], msk[:])
                    row0 = bc * 512 + t * P
                    nc.sync.dma_start(out=cout[row0 : row0 + P, :], in_=cc[:])
    split_excess_waits(nc)
    return nc


def build_expert_kernel():
    """Per core: one expert. Gather C token rows by index, transpose on PE,
    fp32r GEMM vs resident weight, +bias (K=1 ones-matmul PSUM seed),
    scale rows by gate prob. Out: compact y [C, O]."""
    nc = bass.Bass()
    x = nc.dram_tensor("x", [B, D], f32, kind="ExternalInput")
    w = nc.dram_tensor("w", [D, O], f32, kind="ExternalInput")
    bia = nc.dram_tensor("bias", [1, O], f32, kind="ExternalInput")
    idx = nc.dram_tensor("idx", [C, 1], i32, kind="ExternalInput")
    prob = nc.dram_tensor("prob", [P, C // P], f32, kind="ExternalInput")
    y = nc.dram_tensor("y", [C, O], f32, kind="ExternalOutput")
    NM = C // P

    with TileContext(nc) as tc:
        with (
            tc.tile_pool(name="const", bufs=1) as cpool,
            tc.tile_pool(name="gath", bufs=3) as gpool,
            tc.tile_pool(name="xtp", bufs=2) as xpool,
            tc.tile_pool(name="yout", bufs=2) as ypool,
            tc.tile_pool(name="pst", bufs=4, space="PSUM") as tpool,
            tc.tile_pool(name="psy", bufs=1, space="PSUM") as yppool,
        ):
            ident = cpool.tile([P, P], f32)
            make_identity(nc, ident[:])
            ones_f = cpool.tile([1, P], f32)
            nc.vector.memset(ones_f[:], 1.0)
            ones = cpool.tile([1, P], f32r)
            nc.vector.tensor_copy(ones[:], ones_f[:])
            bias_sb = cpool.tile([1, O], f32r)
            nc.sync.dma_start(out=bias_sb[:], in_=bia[:, :].bitcast(f32r))
            prob_sb = cpool.tile([P, NM], f32)
            nc.sync.dma_start(out=prob_sb[:], in_=prob[:, :])
            wsb = cpool.tile([P, 16, O], f32r)
            nc.sync.dma_start(
                out=wsb[:], in_=w.rearrange("(kt p) o -> p kt o", p=P).bitcast(f32r)
            )

            for m in range(NM):
                it = gpool.tile([P, 1], i32, tag="it")
                nc.sync.dma_start(out=it[:], in_=idx[m * P : (m + 1) * P, :])
                xg = gpool.tile([P, D], f32, tag="xg")
                nc.gpsimd.indirect_dma_start(
                    out=xg[:],
                    out_offset=None,
                    in_=x[:],
                    in_offset=bass.IndirectOffsetOnAxis(ap=it[:, :1], axis=0),
                )
                xts = []
                for k in range(16):
                    t_ps = tpool.tile([P, P], f32, tag="t_ps")
                    nc.tensor.transpose(
                        out=t_ps[:], in_=xg[:, k * P : (k + 1) * P], identity=ident[:]
                    )
                    xt = xpool.tile([P, P], f32r, tag=f"xt{k}")
                    nc.vector.tensor_copy(xt[:], t_ps[:])
                    xts.append(xt)
                yps = []
                for o in range(4):
                    ypo = yppool.tile([P, 512], f32, tag=f"yps{o}", name=f"yps{o}")
                    yps.append(ypo)
                for o in range(4):
                    nc.tensor.matmul(
                        yps[o][:],
                        lhsT=ones[:, :],
                        rhs=bias_sb[:, o * 512 : (o + 1) * 512],
                        start=True,
                        stop=False,
                    )
                for k in range(16):
                    for o in range(4):
                        nc.tensor.matmul(
                            yps[o][:],
                            lhsT=xts[k][:],
                            rhs=wsb[:, k, o * 512 : (o + 1) * 512],
                            start=False,
                            stop=(k == 15),
                        )
                ysb = ypool.tile([P, O], f32, tag="ysb")
                for o in range(4):
                    nc.vector.tensor_scalar_mul(
                        ysb[:, o * 512 : (o + 1) * 512],
                        yps[o][:],
                        prob_sb[:, m : m + 1],
                    )
                nc.sync.dma_start(out=y[m * P : (m + 1) * P, :], in_=ysb[:])
    split_excess_waits(nc)
    return nc


_gate_nc = None
_exp_nc = None


def kernel(x, W_e, b_e, W_g, b_g):
    global _gate_nc, _exp_nc
    x = np.ascontiguousarray(np.asarray(x, dtype=np.float32))
    W_e = np.ascontiguousarray(np.asarray(W_e, dtype=np.float32))
    b_e = np.ascontiguousarray(np.asarray(b_e, dtype=np.float32))
    W_g = np.ascontiguousarray(np.asarray(W_g, dtype=np.float32))
    b_g = np.ascontiguousarray(np.asarray(b_g, dtype=np.float32))

    xT = np.ascontiguousarray(x.T)  # [D, B] layout prep for the gate GEMM
    if _gate_nc is None:
        _gate_nc = build_gate_kernel()
    in_maps = [
        {
            "xt": np.ascontiguousarray(xT[:, i * BS : (i + 1) * BS]),
            "wg": W_g,
            "bg": b_g.reshape(E, 1),
        }
        for i in range(E)
    ]
    res_a = run_bass_kernel_spmd(_gate_nc, in_maps, core_ids=list(range(8)))
    c_full = np.concatenate([r["c"] for r in res_a.results], axis=0)  # [B, E]

    # Host routing bookkeeping: per-expert index lists from device-computed c
    idx_list, prob_list, n_list = [], [], []
    for e in range(E):
        sel = np.nonzero(c_full[:, e] > 0.0)[0].astype(np.int32)
        n = len(sel)
        assert n <= C, f"expert {e} over capacity: {n} > {C}"
        idxp = np.zeros((C, 1), np.int32)
        idxp[:n, 0] = sel
        probp = np.zeros(C, np.float32)
        probp[:n] = c_full[sel, e]
        idx_list.append(idxp)
        prob_list.append(np.ascontiguousarray(probp.reshape(C // P, P).T))
        n_list.append(n)

    if _exp_nc is None:
        _exp_nc = build_expert_kernel()
    in_maps = [
        {
            "x": x,
            "w": np.ascontiguousarray(W_e[e]),
            "bias": b_e[e].reshape(1, O),
            "idx": idx_list[e],
            "prob": prob_list[e],
        }
        for e in range(E)
    ]
    res_b = run_bass_kernel_spmd(_exp_nc, in_maps, core_ids=list(range(8)))

    out = np.zeros((B, O), np.float32)
    for e in range(E):
        n = n_list[e]
        out[idx_list[e][:n, 0]] += res_b.results[e]["y"][:n]
    return out


# revision 9
# speedup vs baseline: 46444.4597x; 46444.4597x over previous
"""MoE top-2 routing kernel for 8 Trainium2 NeuronCores.

Strategy (expert parallelism per the sharding hint):
  Launch A (data-parallel gate): each core computes softmax gate + top-2
    combine weights for its 1024-token slice, on device (fp32 matmul for
    exact top-k selection).
  Host: builds per-expert token index lists from the combine weights
    (routing bookkeeping only - all math stays on device).
  Launch B (expert-parallel): core i owns expert i. Gathers its tokens'
    rows of x by index (indirect DMA), transposes on the PE, runs the
    grouped GEMM against its resident expert weight in fp32r, seeds PSUM
    with the expert bias via a K=1 ones-matmul, scales rows by the gate
    probability, and writes the compact [C, 2048] result.
  Host: scatter-adds each expert's compact output into the full [B, 2048]
    output (each token appears in exactly two experts' lists).
"""

import numpy as np

import concourse.bass as bass
import concourse.mybir as mybir
from concourse.bass_utils import run_bass_kernel_spmd
from concourse.masks import make_identity
from concourse.tile import TileContext

B = 8192
D = 2048
O = 2048
E = 8
P = 128
C = 2304  # per-expert token capacity (18 chunks of 128; actual max load 2193)
BS = B // E  # tokens per core in the gate launch

f32 = mybir.dt.float32
f32r = mybir.dt.float32r
i32 = mybir.dt.int32


MAXW = 1  # this walrus build accepts one sync-wait command per instruction
_wsctr = [0]


def split_excess_waits(nc):
    """Post-pass: any instruction carrying more than MAXW sem-waits gets the
    excess moved onto spliced same-engine NoOps just before it (same-engine
    ge-waits executed earlier are semantically identical)."""
    import bass_rust

    for f in nc.m.functions:
        for blk in f.blocks:
            out = []
            changed = False
            for inst in blk.instructions:
                si = inst.sync_info
                if si is not None and len(si.on_wait) > MAXW:
                    waits = list(si.on_wait)
                    excess, keep = waits[:-MAXW], waits[-MAXW:]
                    for i in range(0, len(excess), MAXW):
                        _wsctr[0] += 1
                        nop = bass_rust.InstNoOp(
                            name=f"WSPLIT-{_wsctr[0]}", ins=[], outs=[]
                        )
                        nop.engine = inst.engine
                        nop.sync_info = mybir.SyncInfo(
                            on_wait=excess[i : i + MAXW], on_update=[]
                        )
                        out.append(nop)
                    inst.sync_info = mybir.SyncInfo(
                        on_wait=keep, on_update=list(si.on_update)
                    )
                    changed = True
                out.append(inst)
            if changed:
                blk.instructions = out


def build_gate_kernel():
    """Per core: gate for its BS-token slice. In: xT slice [D, BS], W_g
    [D, E], b_g [E, 1]. Out: combine weights c [BS, E] (top-2 masked
    softmax probs, zeros elsewhere)."""
    nc = bass.Bass()
    xt = nc.dram_tensor("xt", [D, BS], f32, kind="ExternalInput")
    wg = nc.dram_tensor("wg", [D, E], f32, kind="ExternalInput")
    bg = nc.dram_tensor("bg", [E, 1], f32, kind="ExternalInput")
    cout = nc.dram_tensor("c", [BS, E], f32, kind="ExternalOutput")
    NB = BS // 512

    with TileContext(nc) as tc:
        with (
            tc.tile_pool(name="const", bufs=1) as cpool,
            tc.tile_pool(name="work", bufs=2) as wpool,
            tc.tile_pool(name="psum", bufs=2, space="PSUM") as ppool,
            tc.tile_pool(name="psumg", bufs=4, space="PSUM") as pgpool,
        ):
            ident = cpool.tile([P, P], f32)
            make_identity(nc, ident[:])
            wgs = cpool.tile([P, 16, E], f32)
            nc.sync.dma_start(
                out=wgs[:], in_=wg.rearrange("(kt p) e -> p kt e", p=P)
            )
            bgs = cpool.tile([E, 1], f32)
            nc.sync.dma_start(out=bgs[:], in_=bg[:, :])

            for bc in range(NB):
                xts = wpool.tile([P, 16, 512], f32, tag="xts")
                xt3 = xt[:, bc * 512 : (bc + 1) * 512].rearrange(
                    "(kt p) b -> p kt b", p=P
                )
                for k in range(16):
                    nc.sync.dma_start(out=xts[:, k, :], in_=xt3[:, k, :])
                gt_ps = ppool.tile([E, 512], f32, tag="gt_ps")
                for k in range(16):
                    nc.tensor.matmul(
                        gt_ps[:],
                        lhsT=wgs[:, k, :],
                        rhs=xts[:, k, :],
                        start=(k == 0),
                        stop=(k == 15),
                    )
                gt = wpool.tile([E, 512], f32, tag="gt")
                # copy out of PSUM and add the gate bias (per-partition here)
                nc.vector.tensor_scalar_add(gt[:], gt_ps[:], bgs[:, 0:1])

                for t in range(4):
                    g_ps = pgpool.tile([P, E], f32, tag="g_ps")
                    nc.tensor.transpose(
                        out=g_ps[:],
                        in_=gt[:, t * P : (t + 1) * P],
                        identity=ident[:E, :E],
                    )
                    g = wpool.tile([P, E], f32, tag="g")
                    nc.vector.tensor_copy(g[:], g_ps[:])
                    mx = wpool.tile([P, 8], f32, tag="mx")
                    nc.vector.max(out=mx[:], in_=g[:])
                    nc.vector.tensor_scalar(
                        g[:], g[:], mx[:, 0:1], None, op0=mybir.AluOpType.subtract
                    )
                    nc.scalar.activation(g[:], g[:], mybir.ActivationFunctionType.Exp)
                    s = wpool.tile([P, 1], f32, tag="s")
                    nc.vector.reduce_sum(out=s[:], in_=g[:], axis=mybir.AxisListType.X)
                    r = wpool.tile([P, 1], f32, tag="r")
                    nc.vector.reciprocal(r[:], s[:])
                    nc.vector.tensor_scalar_mul(g[:], g[:], r[:, 0:1])
                    # top-2 mask: prob >= (second largest prob)
                    mx2 = wpool.tile([P, 8], f32, tag="mx2")
                    nc.vector.max(out=mx2[:], in_=g[:])
                    msk = wpool.tile([P, E], f32, tag="msk")
                    nc.vector.tensor_scalar(
                        msk[:], g[:], mx2[:, 1:2], None, op0=mybir.AluOpType.is_ge
                    )
                    cc = wpool.tile([P, E], f32, tag="cc")
                    nc.vector.tensor_mul(cc[:], g[:], msk[:])
                    row0 = bc * 512 + t * P
                    nc.sync.dma_start(out=cout[row0 : row0 + P, :], in_=cc[:])
    split_excess_waits(nc)
    return nc


def build_expert_kernel():
    """Per core: one expert. Gather C token rows by index, transpose on PE,
    fp32r GEMM vs resident weight, +bias (K=1 ones-matmul PSUM seed),
    scale rows by gate prob. Out: compact y [C, O]."""
    nc = bass.Bass()
    x = nc.dram_tensor("x", [B, D], f32, kind="ExternalInput")
    w = nc.dram_tensor("w", [D, O], f32, kind="ExternalInput")
    bia = nc.dram_tensor("bias", [1, O], f32, kind="ExternalInput")
    idx = nc.dram_tensor("idx", [C, 1], i32, kind="ExternalInput")
    prob = nc.dram_tensor("prob", [P, C // P], f32, kind="ExternalInput")
    y = nc.dram_tensor("y", [C, O], f32, kind="ExternalOutput")
    NM = C // P

    with TileContext(nc) as tc:
        with (
            tc.tile_pool(name="const", bufs=1) as cpool,
            tc.tile_pool(name="gath", bufs=3) as gpool,
            tc.tile_pool(name="xtp", bufs=2) as xpool,
            tc.tile_pool(name="yout", bufs=2) as ypool,
            tc.tile_pool(name="pst", bufs=4, space="PSUM") as tpool,
            tc.tile_pool(name="psy", bufs=1, space="PSUM") as yppool,
        ):
            ident = cpool.tile([P, P], f32)
            make_identity(nc, ident[:])
            ones_f = cpool.tile([1, P], f32)
            nc.vector.memset(ones_f[:], 1.0)
            ones = cpool.tile([1, P], f32r)
            nc.vector.tensor_copy(ones[:], ones_f[:])
            bias_sb = cpool.tile([1, O], f32r)
            nc.sync.dma_start(out=bias_sb[:], in_=bia[:, :].bitcast(f32r))
            prob_sb = cpool.tile([P, NM], f32)
            nc.sync.dma_start(out=prob_sb[:], in_=prob[:, :])
            wsb = cpool.tile([P, 16, O], f32r)
            w3 = w.rearrange("(kt p) o -> p kt o", p=P).bitcast(f32r)
            for k in range(16):
                nc.sync.dma_start(out=wsb[:, k, :], in_=w3[:, k, :])

            for m in range(NM):
                it = gpool.tile([P, 1], i32, tag="it")
                nc.sync.dma_start(out=it[:], in_=idx[m * P : (m + 1) * P, :])
                xg = gpool.tile([P, D], f32, tag="xg")
                nc.gpsimd.indirect_dma_start(
                    out=xg[:],
                    out_offset=None,
                    in_=x[:],
                    in_offset=bass.IndirectOffsetOnAxis(ap=it[:, :1], axis=0),
                )
                xts = []
                for k in range(16):
                    t_ps = tpool.tile([P, P], f32, tag="t_ps")
                    nc.tensor.transpose(
                        out=t_ps[:], in_=xg[:, k * P : (k + 1) * P], identity=ident[:]
                    )
                    xt = xpool.tile([P, P], f32r, tag=f"xt{k}")
                    nc.vector.tensor_copy(xt[:], t_ps[:])
                    xts.append(xt)
                yps = []
                for o in range(4):
                    ypo = yppool.tile([P, 512], f32, tag=f"yps{o}", name=f"yps{o}")
                    yps.append(ypo)
                for o in range(4):
                    nc.tensor.matmul(
                        yps[o][:],
                        lhsT=ones[:, :],
                        rhs=bias_sb[:, o * 512 : (o + 1) * 512],
                        start=True,
                        stop=False,
                    )
                for k in range(16):
                    for o in range(4):
                        nc.tensor.matmul(
                            yps[o][:],
                            lhsT=xts[k][:],
                            rhs=wsb[:, k, o * 512 : (o + 1) * 512],
                            start=False,
                            stop=(k == 15),
                        )
                ysb = ypool.tile([P, O], f32, tag="ysb")
                for o in range(4):
                    nc.vector.tensor_scalar_mul(
                        ysb[:, o * 512 : (o + 1) * 512],
                        yps[o][:],
                        prob_sb[:, m : m + 1],
                    )
                for q in range(2):
                    cs = q * (O // 2)
                    ce = cs + O // 2
                    nc.sync.dma_start(
                        out=y[m * P : (m + 1) * P, cs:ce], in_=ysb[:, cs:ce]
                    )
    split_excess_waits(nc)
    return nc


_gate_nc = None
_exp_nc = None


def kernel(x, W_e, b_e, W_g, b_g):
    global _gate_nc, _exp_nc
    x = np.ascontiguousarray(np.asarray(x, dtype=np.float32))
    W_e = np.ascontiguousarray(np.asarray(W_e, dtype=np.float32))
    b_e = np.ascontiguousarray(np.asarray(b_e, dtype=np.float32))
    W_g = np.ascontiguousarray(np.asarray(W_g, dtype=np.float32))
    b_g = np.ascontiguousarray(np.asarray(b_g, dtype=np.float32))

    xT = np.ascontiguousarray(x.T)  # [D, B] layout prep for the gate GEMM
    if _gate_nc is None:
        _gate_nc = build_gate_kernel()
    in_maps = [
        {
            "xt": np.ascontiguousarray(xT[:, i * BS : (i + 1) * BS]),
            "wg": W_g,
            "bg": b_g.reshape(E, 1),
        }
        for i in range(E)
    ]
    res_a = run_bass_kernel_spmd(_gate_nc, in_maps, core_ids=list(range(8)))
    c_full = np.concatenate([r["c"] for r in res_a.results], axis=0)  # [B, E]

    # Host routing bookkeeping: per-expert index lists from device-computed c
    idx_list, prob_list, n_list = [], [], []
    for e in range(E):
        sel = np.nonzero(c_full[:, e] > 0.0)[0].astype(np.int32)
        n = len(sel)
        assert n <= C, f"expert {e} over capacity: {n} > {C}"
        idxp = np.zeros((C, 1), np.int32)
        idxp[:n, 0] = sel
        probp = np.zeros(C, np.float32)
        probp[:n] = c_full[sel, e]
        idx_list.append(idxp)
        prob_list.append(np.ascontiguousarray(probp.reshape(C // P, P).T))
        n_list.append(n)

    if _exp_nc is None:
        _exp_nc = build_expert_kernel()
    in_maps = [
        {
            "x": x,
            "w": np.ascontiguousarray(W_e[e]),
            "bias": b_e[e].reshape(1, O),
            "idx": idx_list[e],
            "prob": prob_list[e],
        }
        for e in range(E)
    ]
    res_b = run_bass_kernel_spmd(_exp_nc, in_maps, core_ids=list(range(8)))

    out = np.zeros((B, O), np.float32)
    for e in range(E):
        n = n_list[e]
        out[idx_list[e][:n, 0]] += res_b.results[e]["y"][:n]
    return out
